# revision 8
# baseline (speedup 1.0000x reference)
"""Trainium2 kernel for cellpose-style flow integration (grid_sample scan).

Strategy:
  - Host builds a padded "patch table" T[r*2050+c] = the 8 values
    [a00,a01,a10,a11,b00,b01,b10,b11] of the 2x2 bilinear corner patch at
    padded pixel (r,c), PRE-SCALED by 1024 so the integration state can be
    kept directly in padded pixel coordinates u = pt*1024 + 1024.5.
    Zero padding rows/cols encode grid_sample's zeros-padding.
  - Points are sharded across 8 NeuronCores (32768 each, laid out [128,256]).
  - Each iteration, per chunk of 128 free columns: floor/frac via the
    round-to-nearest int convert (floor(u) = i32(u - 0.5); the only miss is
    exact-integer u, where ties-to-even may give floor-1 with fx = 1.0 —
    which the bilinear lerp then resolves to the right corner exactly),
    q = yf*2050 + xf, 128 per-partition indirect-DMA patch gathers from
    HBM (the HW indirect-DMA ucode does exactly 128 rows — one offset per
    partition — per instruction at ~1.1us/instruction back-to-back; the
    Pool engine's per-instruction descriptor generation is the kernel's
    bottleneck), separable bilinear lerp on DVE, u += s, clamp to
    [0.5, 2048.5].  Chunked so chunk A's gathers overlap chunk B's
    vector ops.
"""
import numpy as np

H = W = 2048
NPTS = 262144
N_CORES = 8
PTS_PER_CORE = NPTS // N_CORES          # 32768
P = 128
F = PTS_PER_CORE // P                   # 256 free elems per partition
PAD = 2050                              # padded table row length
NCHUNK = 2

_compiled = {}


def _build_nc(niter: int):
    import concourse.bass as bass
    import concourse.mybir as mybir
    import concourse.tile as tile
    from concourse import bacc

    f32 = mybir.dt.float32
    i32 = mybir.dt.int32
    Alu = mybir.AluOpType

    nc = bacc.Bacc("TRN2", target_bir_lowering=False, debug=False,
                   num_devices=N_CORES)
    tab = nc.dram_tensor("tab", [PAD * PAD, 8], f32, kind="ExternalInput").ap()
    p0x = nc.dram_tensor("p0x", [P, F], f32, kind="ExternalInput").ap()
    p0y = nc.dram_tensor("p0y", [P, F], f32, kind="ExternalInput").ap()
    outx = nc.dram_tensor("outx", [P, F], f32, kind="ExternalOutput").ap()
    outy = nc.dram_tensor("outy", [P, F], f32, kind="ExternalOutput").ap()

    FC = F // NCHUNK

    with tile.TileContext(nc) as tc:
        with (
            tc.tile_pool(name="state", bufs=1) as state,
            tc.tile_pool(name="scratch", bufs=3) as scratch,
            tc.tile_pool(name="gbuf", bufs=3) as gbuf,
        ):
            ux = state.tile([P, F], f32, tag="ux")
            uy = state.tile([P, F], f32, tag="uy")
            nc.gpsimd.dma_start(out=ux[:], in_=p0x[:])
            nc.gpsimd.dma_start(out=uy[:], in_=p0y[:])

            for it in range(niter):
                for c in range(NCHUNK):
                    cs = slice(c * FC, (c + 1) * FC)
                    uxc = ux[:, cs]
                    uyc = uy[:, cs]

                    fx = scratch.tile([P, FC], f32, tag="fx")
                    fy = scratch.tile([P, FC], f32, tag="fy")
                    xf = scratch.tile([P, FC], f32, tag="xf")
                    yf = scratch.tile([P, FC], f32, tag="yf")
                    qf = scratch.tile([P, FC], f32, tag="qf")
                    qi = scratch.tile([P, FC], i32, tag="qi")
                    ti = scratch.tile([P, FC], i32, tag="ti")
                    t = scratch.tile([P, FC], f32, tag="t")

                    # floor(u) = i32(u - 0.5) (round-to-nearest convert);
                    # fr = u - floor(u)
                    for (uc, fr, fl) in ((uxc, fx, xf), (uyc, fy, yf)):
                        nc.vector.tensor_scalar(out=t[:], in0=uc, scalar1=0.5,
                                                scalar2=None, op0=Alu.subtract)
                        nc.vector.tensor_copy(out=ti[:], in_=t[:])
                        nc.vector.tensor_copy(out=fl[:], in_=ti[:])
                        nc.vector.tensor_tensor(out=fr[:], in0=uc, in1=fl[:],
                                                op=Alu.subtract)
                    # qf = yf * 2050 + xf  (exact in f32: < 2^23)
                    nc.vector.scalar_tensor_tensor(out=qf[:], in0=yf[:],
                                                   scalar=2050.0, in1=xf[:],
                                                   op0=Alu.mult, op1=Alu.add)
                    nc.vector.tensor_copy(out=qi[:], in_=qf[:])

                    g = gbuf.tile([P, FC, 8], f32, tag="g")
                    for j in range(FC):
                        nc.gpsimd.indirect_dma_start(
                            out=g[:, j, :],
                            out_offset=None,
                            in_=tab[:, :],
                            in_offset=bass.IndirectOffsetOnAxis(
                                ap=qi[:, j:j + 1], axis=0),
                        )

                    # x-lerp: h = g_even + fx * (g_odd - g_even)
                    d = scratch.tile([P, FC, 4], f32, tag="d")
                    h = scratch.tile([P, FC, 4], f32, tag="h")
                    nc.vector.tensor_tensor(out=d[:], in0=g[:, :, 1::2],
                                            in1=g[:, :, 0::2], op=Alu.subtract)
                    nc.vector.tensor_tensor(out=d[:], in0=d[:],
                                            in1=fx[:].to_broadcast([P, FC, 4]),
                                            op=Alu.mult)
                    nc.vector.tensor_tensor(out=h[:], in0=g[:, :, 0::2],
                                            in1=d[:], op=Alu.add)
                    # y-lerp: s = h_even + fy * (h_odd - h_even)
                    d2 = scratch.tile([P, FC, 2], f32, tag="d2")
                    s = scratch.tile([P, FC, 2], f32, tag="s")
                    nc.vector.tensor_tensor(out=d2[:], in0=h[:, :, 1::2],
                                            in1=h[:, :, 0::2], op=Alu.subtract)
                    nc.vector.tensor_tensor(out=d2[:], in0=d2[:],
                                            in1=fy[:].to_broadcast([P, FC, 2]),
                                            op=Alu.mult)
                    nc.vector.tensor_tensor(out=s[:], in0=h[:, :, 0::2],
                                            in1=d2[:], op=Alu.add)

                    # u += s ; clamp to [0.5, 2048.5]
                    for (uc, k) in ((uxc, 0), (uyc, 1)):
                        nc.vector.tensor_tensor(out=uc, in0=uc, in1=s[:, :, k],
                                                op=Alu.add)
                        nc.vector.tensor_scalar(out=uc, in0=uc, scalar1=0.5,
                                                scalar2=2048.5, op0=Alu.max,
                                                op1=Alu.min)

            # final: pix = ((u - 1024.5) / 1024 + 1) * 1023.5
            ox = state.tile([P, F], f32, tag="ox")
            oy = state.tile([P, F], f32, tag="oy")
            for (u, o) in ((ux, ox), (uy, oy)):
                nc.vector.tensor_scalar(out=o[:], in0=u[:], scalar1=1024.5,
                                        scalar2=1.0 / 1024.0,
                                        op0=Alu.subtract, op1=Alu.mult)
                nc.vector.tensor_scalar(out=o[:], in0=o[:], scalar1=1.0,
                                        scalar2=1023.5, op0=Alu.add,
                                        op1=Alu.mult)
            nc.gpsimd.dma_start(out=outx[:], in_=ox[:])
            nc.gpsimd.dma_start(out=outy[:], in_=oy[:])

    nc.compile()
    return nc


def _build_table(dP: np.ndarray) -> np.ndarray:
    """T[r*2050+c, 0:8] = 2x2 patch of (im0,im1)*1024 at padded (r,c)."""
    scale = np.float32(2.0 / 2047.0)
    im0 = (dP[1] * scale).astype(np.float32) * np.float32(1024.0)  # adds to x
    im1 = (dP[0] * scale).astype(np.float32) * np.float32(1024.0)  # adds to y
    imp = np.zeros((PAD + 1, PAD + 1, 2), np.float32)
    imp[1:H + 1, 1:W + 1, 0] = im0
    imp[1:H + 1, 1:W + 1, 1] = im1
    T = np.empty((PAD, PAD, 8), np.float32)
    T[:, :, 0] = imp[:PAD, :PAD, 0]       # a00
    T[:, :, 1] = imp[:PAD, 1:, 0]         # a01
    T[:, :, 2] = imp[1:, :PAD, 0]         # a10
    T[:, :, 3] = imp[1:, 1:, 0]           # a11
    T[:, :, 4] = imp[:PAD, :PAD, 1]       # b00
    T[:, :, 5] = imp[:PAD, 1:, 1]         # b01
    T[:, :, 6] = imp[1:, :PAD, 1]         # b10
    T[:, :, 7] = imp[1:, 1:, 1]           # b11
    return T.reshape(PAD * PAD, 8)


def _initial_pts(inds: np.ndarray):
    """Initial padded pixel coords u = pt*1024 + 1024.5, pt in [-1,1]."""
    f = np.float32
    sizes = f(2047.0)
    ptx = inds[1].astype(f) / sizes * f(2.0) - f(1.0)
    pty = inds[0].astype(f) / sizes * f(2.0) - f(1.0)
    ux = ptx * f(1024.0) + f(1024.5)
    uy = pty * f(1024.0) + f(1024.5)
    return ux, uy


def kernel(dP: np.ndarray, inds: np.ndarray, niter) -> np.ndarray:
    from concourse.bass_utils import run_bass_kernel_spmd

    niter = int(niter)
    dP = np.asarray(dP, np.float32)
    inds = np.asarray(inds)

    if niter not in _compiled:
        _compiled[niter] = _build_nc(niter)
    nc = _compiled[niter]

    T = _build_table(dP)
    ptx, pty = _initial_pts(inds)

    in_maps = []
    for i in range(N_CORES):
        sl = slice(i * PTS_PER_CORE, (i + 1) * PTS_PER_CORE)
        in_maps.append({
            "tab": T,
            "p0x": ptx[sl].reshape(P, F),
            "p0y": pty[sl].reshape(P, F),
        })

    res = run_bass_kernel_spmd(nc, in_maps, list(range(N_CORES)))

    out = np.empty((2, NPTS), np.float32)
    for i in range(N_CORES):
        sl = slice(i * PTS_PER_CORE, (i + 1) * PTS_PER_CORE)
        out[0, sl] = res.results[i]["outy"].reshape(-1)
        out[1, sl] = res.results[i]["outx"].reshape(-1)
    return out
